# revision 26
# baseline (speedup 1.0000x reference)
"""Feature-pyramid ROIAlign (multi-level crop) on 8 TRN2 NeuronCores — v6.

Host routes each proposal to a pyramid level and a window bucket
(5x5 / 8x8 / 11x11 cells, smallest covering its bilinear support) and
builds per-proposal dense interpolation matrices [w*w, 196] bf16.

Device (one SPMD graph; structure = per-(region,bucket) counts, all
offsets/weights are runtime data):
  - patch gather: gpsimd dma_gather per arena region (int16 row indices,
    one 512B cell-row per index) filling 128-partition stripes. Stripes
    pack 3/2/1 slots at PE-quadrant partition bases {0,32,64} for
    k = 25/64/121.
  - per slot: two k-row bf16 matmuls (channel halves) into PSUM, reading
    slab + weight tiles at the slot's partition base.
  - PSUM->SBUF f32->bf16 casts alternate Vector/Scalar engines.
  - bf16 output written per 8-stripe group via SP/ACT DMAs.
"""
import os
import numpy as np
import ml_dtypes

RPN_SCALES = (2.0, 4.0, 8.0, 16.0)
BASE_SIZES = (8.0, 16.0, 32.0, 64.0)
S = 14
S2 = S * S
PWMAX = 11
C = 256
MAP_HW = (256, 128, 64, 32)
ARENA_BASE = (0, 65536, 81920, 86016)
ARENA_ROWS = 87040
N_CORES = 8
BUCKETS = (5, 8, 11)
SPB = {5: 3, 8: 2, 11: 1}            # slots per 128-partition stripe
SLOT_OFS = {5: (0, 32, 64), 8: (0, 64), 11: (0,)}
G_ST = 8                              # stripes per weight/output group
MAX_STR_PER_GATHER = 8   # >8 (1024 idx) wedges the SWDGE ring
REGION_W = 32768                      # int16 index window (rows)

LAST_EXEC_TIME_NS = None
_GRAPH_CACHE = {}


def _route_and_weights(proposals):
    p = proposals.astype(np.float32)
    x0, y0, x1, y1 = p[:, 1], p[:, 2], p[:, 3], p[:, 4]
    sizes = np.sqrt((x1 - x0) * (y1 - y0))
    base = np.asarray(BASE_SIZES, dtype=np.float32)
    lvl = np.argmin(np.abs(sizes[:, None] - base[None, :]), axis=1).astype(
        np.int32)

    N = p.shape[0]
    stride = np.asarray(RPN_SCALES, dtype=np.float32)[lvl]
    M = np.asarray(MAP_HW, dtype=np.int32)[lvl]

    fx0, fy0, fx1, fy1 = (c / stride for c in (x0, y0, x1, y1))
    bw = (fx1 - fx0) / np.float32(S)
    bh = (fy1 - fy0) / np.float32(S)
    grid = np.arange(S, dtype=np.float32) + np.float32(0.5)
    xs = fx0[:, None] + grid[None, :] * bw[:, None] - np.float32(0.5)
    ys = fy0[:, None] + grid[None, :] * bh[:, None] - np.float32(0.5)

    def split(coord, Mv):
        c0 = np.floor(coord)
        frac = coord - c0
        i0 = np.clip(c0.astype(np.int64), 0, Mv - 1).astype(np.int32)
        i1 = np.minimum(i0 + 1, Mv - 1).astype(np.int32)
        return i0, i1, frac.astype(np.float32)

    Mv = M[:, None]
    yi0, yi1, wy = split(ys, Mv)
    xi0, xi1, wx = split(xs, Mv)

    span = np.maximum(yi1.max(axis=1) - yi0.min(axis=1),
                      xi1.max(axis=1) - xi0.min(axis=1)) + 1
    assert span.max() <= PWMAX, "proposal spans >11 feature cells"
    wbuck = np.full(N, BUCKETS[-1], dtype=np.int32)
    for w in reversed(BUCKETS):
        wbuck[span <= w] = w

    oy = np.clip(yi0.min(axis=1), 0, M - wbuck)
    ox = np.clip(xi0.min(axis=1), 0, M - wbuck)
    ly0, ly1 = yi0 - oy[:, None], yi1 - oy[:, None]
    lx0, lx1 = xi0 - ox[:, None], xi1 - ox[:, None]
    assert ly0.min() >= 0 and lx0.min() >= 0
    assert (ly1.max(axis=1) < wbuck).all() and (lx1.max(axis=1) < wbuck).all()

    ii = np.arange(S)
    nn = np.arange(N)[:, None]
    Wy = np.zeros((N, S, PWMAX), dtype=np.float32)
    Wx = np.zeros((N, S, PWMAX), dtype=np.float32)
    np.add.at(Wy, (nn, ii[None, :], ly0), 1.0 - wy)
    np.add.at(Wy, (nn, ii[None, :], ly1), wy)
    np.add.at(Wx, (nn, ii[None, :], lx0), 1.0 - wx)
    np.add.at(Wx, (nn, ii[None, :], lx1), wx)

    # per-proposal first arena row and region base
    ab = np.asarray(ARENA_BASE, dtype=np.int64)[lvl]
    row0 = ab + oy.astype(np.int64) * M + ox
    region = np.where(lvl > 0, np.int64(ARENA_BASE[1]),
                      np.minimum(row0 // 16384, 2) * 16384)
    assert (row0 - region >= 0).all()
    assert (row0 - region + (wbuck - 1) * M.astype(np.int64)
            + wbuck - 1 < REGION_W).all()

    wfull = {}
    cls_pos = np.zeros(N, dtype=np.int64)
    for w in BUCKETS:
        ids = np.where(wbuck == w)[0]
        cls_pos[ids] = np.arange(len(ids))
        if len(ids) == 0:
            wfull[w] = np.zeros((0, w * w, S2), dtype=ml_dtypes.bfloat16)
            continue
        wf = np.einsum("niy,njx->nyxij", Wy[ids, :, :w], Wx[ids, :, :w])
        wfull[w] = wf.reshape(len(ids), w * w, S2).astype(ml_dtypes.bfloat16)
    return lvl, wbuck, cls_pos, region, oy, ox, wfull


def _shard(wbuck, region):
    """Round-robin each (region, bucket) class across cores (pad to x8).
    Returns slot_gid [N_CORES, M] and class key tuple."""
    keys = sorted(set(zip(region.tolist(), wbuck.tolist())))
    slot_gid = [[] for _ in range(N_CORES)]
    classes = []
    for r, w in keys:
        ids = np.where((region == r) & (wbuck == w))[0]
        pad = (-len(ids)) % N_CORES
        if pad:
            ids = np.concatenate([ids, np.repeat(ids[-1], pad)])
        per = len(ids) // N_CORES
        for k in range(N_CORES):
            slot_gid[k].extend(ids[k::N_CORES].tolist())
        classes.append((int(r), int(w), per))
    return np.asarray(slot_gid, dtype=np.int64), tuple(classes)


def _plan(classes):
    """Per-core static schedule.
    stripes: (w, region, slot_start, slot_cnt)
    gathers: (region, stripe_start, n_stripes)
    groups:  (stripe_start, n_stripes, slot_start, slot_cnt)"""
    stripes = []
    slot = 0
    for r, w, per in classes:
        left = per
        while left > 0:
            cnt = min(SPB[w], left)
            stripes.append((w, r, slot, cnt))
            slot += cnt
            left -= cnt
    M = slot
    gathers = []
    i = 0
    while i < len(stripes):
        r = stripes[i][1]
        j = i
        while (j < len(stripes) and stripes[j][1] == r
               and j - i < MAX_STR_PER_GATHER):
            j += 1
        gathers.append((r, i, j - i))
        i = j
    groups = []
    for a in range(0, len(stripes), G_ST):
        b = min(a + G_ST, len(stripes))
        s0 = stripes[a][2]
        s1 = stripes[b - 1][2] + stripes[b - 1][3]
        groups.append((a, b - a, s0, s1 - s0))
    return stripes, gathers, groups, M


def _build_graph(classes):
    import concourse.bass as bass
    import concourse.bacc as bacc
    import concourse.mybir as mybir
    import concourse.tile as tile

    stripes, gathers, groups, M = _plan(classes)
    S_tot = len(stripes)
    IDXC = len(gathers) * MAX_STR_PER_GATHER * 8  # 64B-aligned gather blocks

    nc = bacc.Bacc()
    arena = nc.declare_dram_parameter("arena", [ARENA_ROWS, C],
                                      mybir.dt.bfloat16, isOutput=False)
    idxp = nc.declare_dram_parameter("idxp", [128, IDXC],
                                     mybir.dt.int16, isOutput=False)
    wmat = nc.declare_dram_parameter("wmat", [128, S_tot * S2],
                                     mybir.dt.bfloat16, isOutput=False)
    out = nc.declare_dram_parameter("out", [C, M, S2], mybir.dt.bfloat16,
                                    isOutput=True)

    with tile.TileContext(nc) as tc:
        with (
            tc.tile_pool(name="small", bufs=1) as psmall,
            tc.tile_pool(name="slabp", bufs=1) as pslab,
            tc.tile_pool(name="wpool", bufs=3) as pwp,
            tc.tile_pool(name="outp", bufs=3) as po,
            tc.tile_pool(name="ps", bufs=4, space="PSUM") as ppsum,
        ):
            idx_t = psmall.tile([128, IDXC], mybir.dt.int16)
            nc.sync.dma_start(idx_t[:], idxp[:])

            # warm-up gather (zero indices, result unused): absorbs the
            # one-time SWDGE gather setup while idx_t is still in flight
            warm_i = psmall.tile([128, 8], mybir.dt.int16)
            nc.vector.memset(warm_i[:], 0)
            warm_o = psmall.tile([128, C], mybir.dt.bfloat16)
            nc.gpsimd.dma_gather(
                out_ap=warm_o[:].rearrange("p (j c) -> p j c", j=1),
                in_ap=arena[0:1024, :], idxs_ap=warm_i[:],
                num_idxs=128, num_idxs_reg=128, elem_size=C)

            slabs = []        # per gather: (tile, stripe_start)
            for gi, (r, st0, n_str) in enumerate(gathers):
                sl = pslab.tile([128, n_str * C], mybir.dt.bfloat16,
                                tag=f"sl{gi}", name=f"slab_{gi}")
                hi = min(r + REGION_W, ARENA_ROWS)
                c0 = gi * MAX_STR_PER_GATHER * 8
                nc.gpsimd.dma_gather(
                    out_ap=sl[:].rearrange("p (j c) -> p j c", j=n_str),
                    in_ap=arena[r:hi, :],
                    idxs_ap=idx_t[:, c0:c0 + n_str * 8],
                    num_idxs=n_str * 128,
                    num_idxs_reg=n_str * 128,
                    elem_size=C,
                )
                slabs.append((sl, st0))

            def stripe_slab(si):
                for sl, st0 in reversed(slabs):
                    if si >= st0:
                        return sl, si - st0
                raise AssertionError

            def emit_wt(gi):
                a, n_str, s0, n_slots = groups[gi]
                wt = pwp.tile([128, n_str * S2], mybir.dt.bfloat16,
                              tag="wt", name=f"wt_{gi}")
                nc.sync.dma_start(wt[:], wmat[:, a * S2:(a + n_str) * S2])
                return wt

            cast_rr = 0
            wt_next = emit_wt(0)
            for gi, (a, n_str, s0, n_slots) in enumerate(groups):
                wt = wt_next
                if gi + 1 < len(groups):
                    wt_next = emit_wt(gi + 1)
                outAB = po.tile([128, 2 * n_slots * S2], mybir.dt.bfloat16,
                                tag="outAB", name=f"outAB_{gi}")
                # per-slot matmul args within this group
                sargs = []    # (slab, slab_col, wt_col, part_ofs, k)
                for si in range(a, a + n_str):
                    w, r, sst, scnt = stripes[si]
                    sl, j = stripe_slab(si)
                    for q in range(scnt):
                        sargs.append((sl, j * C, (si - a) * S2,
                                      SLOT_OFS[w][q], w * w))
                # one slot per PSUM tile: matmuls with different PE tile
                # positions must not share a PSUM tile (HW wedge)
                for q0 in range(n_slots):
                    psAB = ppsum.tile([128, 1024], mybir.dt.float32,
                                      tag="psAB", name=f"ps_{gi}_{q0}")
                    sl, scol, wcol, o, k = sargs[q0]
                    nc.tensor.matmul(psAB[:, 0:S2],
                                     sl[o:o + k, scol:scol + 128],
                                     wt[o:o + k, wcol:wcol + S2],
                                     start=True, stop=True)
                    nc.tensor.matmul(psAB[:, 512:512 + S2],
                                     sl[o:o + k, scol + 128:scol + C],
                                     wt[o:o + k, wcol:wcol + S2],
                                     start=True, stop=True)
                    src = psAB[:].rearrange("p (b x) -> p b x", b=2)[
                        :, :, 0:S2]
                    dst = outAB[:].rearrange("p (b x) -> p b x", b=2)[
                        :, :, q0 * S2:(q0 + 1) * S2]
                    if cast_rr % 2 == 0:
                        nc.vector.tensor_copy(dst, src)
                    else:
                        nc.scalar.copy(dst, src)
                    cast_rr += 1
                nc.sync.dma_start(out[0:128, s0:s0 + n_slots, :],
                                  outAB[:, 0:n_slots * S2])
                nc.scalar.dma_start(out[128:256, s0:s0 + n_slots, :],
                                    outAB[:, n_slots * S2:2 * n_slots * S2])
    nc.finalize()
    return nc


def _prep_core_inputs(k, slot_gid, classes, lvl, cls_pos, region, oy, ox,
                      wfull):
    stripes, gathers, groups, M = _plan(classes)
    S_tot = len(stripes)
    slots = slot_gid[k]
    mw = np.asarray(MAP_HW, dtype=np.int64)
    ab = np.asarray(ARENA_BASE, dtype=np.int64)

    idx = np.zeros((S_tot, 128), dtype=np.int16)
    wblob = np.zeros((128, S_tot * S2), dtype=ml_dtypes.bfloat16)
    for si, (w, r, sst, scnt) in enumerate(stripes):
        K = w * w
        dyx = (np.repeat(np.arange(w), w)[None, :]
               * mw[lvl[slots[sst:sst + scnt]]][:, None]
               + np.tile(np.arange(w), w)[None, :])          # [scnt, K]
        gids = slots[sst:sst + scnt]
        row = (ab[lvl[gids]] + oy[gids].astype(np.int64) * mw[lvl[gids]]
               + ox[gids])[:, None] + dyx - r
        assert row.min() >= 0 and row.max() < REGION_W
        for q in range(scnt):
            o = SLOT_OFS[w][q]
            idx[si, o:o + K] = row[q].astype(np.int16)
            wblob[o:o + K, si * S2:(si + 1) * S2] = wfull[w][cls_pos[gids[q]]]

    # per-gather 64B-aligned int16 blocks: gather gi occupies cols
    # [gi*MAX*8, gi*MAX*8 + n_str*8); within a block index i at
    # [i%16, i//16], replicated 8x down 128 partitions
    IDXC = len(gathers) * MAX_STR_PER_GATHER * 8
    blk = np.zeros((16, IDXC), dtype=np.int16)
    for gi, (r, st0, n_str) in enumerate(gathers):
        c0 = gi * MAX_STR_PER_GATHER * 8
        flat = idx[st0:st0 + n_str].reshape(-1)              # [n_str*128]
        blk[:, c0:c0 + n_str * 8] = flat.reshape(n_str * 8, 16).T
    idxp = np.ascontiguousarray(np.tile(blk, (8, 1)))
    return idxp, wblob


def _install_profile_hook():
    import contextlib
    import ctypes
    import sys
    import types
    if "antenv.axon_hooks" in sys.modules:
        return
    so_path = "/opt/axon/libaxon_pjrt.so"
    try:
        lib = ctypes.CDLL(so_path)
        lib.axon_start_nrt_profile.argtypes = [
            ctypes.POINTER(ctypes.c_int64), ctypes.c_size_t]
        lib.axon_start_nrt_profile.restype = ctypes.c_int64
        lib.axon_stop_nrt_profile.argtypes = [ctypes.c_char_p]
        lib.axon_stop_nrt_profile.restype = ctypes.c_int64
    except (OSError, AttributeError):
        return

    @contextlib.contextmanager
    def _hook(output_dir, device_ids):
        import jax
        jax.devices()
        if device_ids:
            ids = (ctypes.c_int64 * len(device_ids))(*device_ids)
            rc = lib.axon_start_nrt_profile(ids, len(device_ids))
        else:
            rc = lib.axon_start_nrt_profile(None, 0)
        if rc != 0:
            raise RuntimeError(f"axon_start_nrt_profile rc={rc}")
        try:
            yield
        finally:
            n = lib.axon_stop_nrt_profile(str(output_dir).encode())
            if n < 0:
                raise RuntimeError(f"axon_stop_nrt_profile rc={n}")

    mod = types.ModuleType("antenv.axon_hooks")
    mod.get_axon_ntff_profile_hook = lambda: _hook
    mod.set_axon_ntff_profile_hook = lambda h: None
    sys.modules["antenv.axon_hooks"] = mod
    try:
        import antenv
        antenv.axon_hooks = mod
    except ImportError:
        pass


def kernel(f0, f1, f2, f3, proposals):
    global LAST_EXEC_TIME_NS
    try:
        _install_profile_hook()
    except Exception:
        pass
    from concourse.bass_utils import run_bass_kernel_spmd

    feats = (f0, f1, f2, f3)
    N = proposals.shape[0]
    lvl, wbuck, cls_pos, region, oy, ox, wfull = _route_and_weights(
        np.asarray(proposals))
    slot_gid, classes = _shard(wbuck, region)

    if classes not in _GRAPH_CACHE:
        _GRAPH_CACHE[classes] = _build_graph(classes)
    nc = _GRAPH_CACHE[classes]

    arena_np = np.concatenate([
        np.ascontiguousarray(np.asarray(f)[0].transpose(1, 2, 0)).astype(
            ml_dtypes.bfloat16).reshape(-1, C)
        for f in feats
    ], axis=0)
    assert arena_np.shape[0] == ARENA_ROWS

    in_maps = []
    for k in range(N_CORES):
        idxp, wblob = _prep_core_inputs(k, slot_gid, classes, lvl, cls_pos,
                                        region, oy, ox, wfull)
        in_maps.append({"arena": arena_np, "idxp": idxp, "wmat": wblob})

    trace = os.environ.get("KERNEL_TRACE", "0") == "1"
    res = run_bass_kernel_spmd(nc, in_maps, list(range(N_CORES)), trace=trace)
    LAST_EXEC_TIME_NS = res.exec_time_ns

    out_full = np.zeros((N, C, S2), dtype=np.float32)
    for k in range(N_CORES):
        out_full[slot_gid[k]] = res.results[k]["out"].astype(
            np.float32).transpose(1, 0, 2)
    return out_full.reshape(N, C, S, S)


# revision 30
# speedup vs baseline: 1.0844x; 1.0844x over previous
"""Feature-pyramid ROIAlign (multi-level crop) on 8 TRN2 NeuronCores — v6.

Host routes each proposal to a pyramid level and a window bucket
(5x5 / 8x8 / 11x11 cells, smallest covering its bilinear support) and
builds per-proposal dense interpolation matrices [w*w, 196] bf16.

Device (one SPMD graph; structure = per-(region,bucket) counts, all
offsets/weights are runtime data):
  - patch gather: gpsimd dma_gather per arena region (int16 row indices,
    one 512B cell-row per index) filling 128-partition stripes. Stripes
    pack 3/2/1 slots at PE-quadrant partition bases {0,32,64} for
    k = 25/64/121.
  - per slot: two k-row bf16 matmuls (channel halves) into PSUM, reading
    slab + weight tiles at the slot's partition base.
  - PSUM->SBUF f32->bf16 casts alternate Vector/Scalar engines.
  - bf16 output written per 8-stripe group via SP/ACT DMAs.
"""
import os
import numpy as np
import ml_dtypes

RPN_SCALES = (2.0, 4.0, 8.0, 16.0)
BASE_SIZES = (8.0, 16.0, 32.0, 64.0)
S = 14
S2 = S * S
PWMAX = 11
C = 256
MAP_HW = (256, 128, 64, 32)
ARENA_BASE = (0, 65536, 81920, 86016)
ARENA_ROWS = 87040
N_CORES = 8
BUCKETS = (5, 8, 11)
SPB = {5: 3, 8: 2, 11: 1}            # slots per 128-partition stripe
SLOT_OFS = {5: (0, 32, 64), 8: (0, 64), 11: (0,)}
G_ST = 8                              # stripes per weight/output group
MAX_STR_PER_GATHER = 8   # >8 (1024 idx) wedges the SWDGE ring
REGION_W = 32768                      # int16 index window (rows)

LAST_EXEC_TIME_NS = None
_GRAPH_CACHE = {}


def _route_and_weights(proposals):
    p = proposals.astype(np.float32)
    x0, y0, x1, y1 = p[:, 1], p[:, 2], p[:, 3], p[:, 4]
    sizes = np.sqrt((x1 - x0) * (y1 - y0))
    base = np.asarray(BASE_SIZES, dtype=np.float32)
    lvl = np.argmin(np.abs(sizes[:, None] - base[None, :]), axis=1).astype(
        np.int32)

    N = p.shape[0]
    stride = np.asarray(RPN_SCALES, dtype=np.float32)[lvl]
    M = np.asarray(MAP_HW, dtype=np.int32)[lvl]

    fx0, fy0, fx1, fy1 = (c / stride for c in (x0, y0, x1, y1))
    bw = (fx1 - fx0) / np.float32(S)
    bh = (fy1 - fy0) / np.float32(S)
    grid = np.arange(S, dtype=np.float32) + np.float32(0.5)
    xs = fx0[:, None] + grid[None, :] * bw[:, None] - np.float32(0.5)
    ys = fy0[:, None] + grid[None, :] * bh[:, None] - np.float32(0.5)

    def split(coord, Mv):
        c0 = np.floor(coord)
        frac = coord - c0
        i0 = np.clip(c0.astype(np.int64), 0, Mv - 1).astype(np.int32)
        i1 = np.minimum(i0 + 1, Mv - 1).astype(np.int32)
        return i0, i1, frac.astype(np.float32)

    Mv = M[:, None]
    yi0, yi1, wy = split(ys, Mv)
    xi0, xi1, wx = split(xs, Mv)

    span = np.maximum(yi1.max(axis=1) - yi0.min(axis=1),
                      xi1.max(axis=1) - xi0.min(axis=1)) + 1
    assert span.max() <= PWMAX, "proposal spans >11 feature cells"
    wbuck = np.full(N, BUCKETS[-1], dtype=np.int32)
    for w in reversed(BUCKETS):
        wbuck[span <= w] = w

    oy = np.clip(yi0.min(axis=1), 0, M - wbuck)
    ox = np.clip(xi0.min(axis=1), 0, M - wbuck)
    ly0, ly1 = yi0 - oy[:, None], yi1 - oy[:, None]
    lx0, lx1 = xi0 - ox[:, None], xi1 - ox[:, None]
    assert ly0.min() >= 0 and lx0.min() >= 0
    assert (ly1.max(axis=1) < wbuck).all() and (lx1.max(axis=1) < wbuck).all()

    ii = np.arange(S)
    nn = np.arange(N)[:, None]
    Wy = np.zeros((N, S, PWMAX), dtype=np.float32)
    Wx = np.zeros((N, S, PWMAX), dtype=np.float32)
    np.add.at(Wy, (nn, ii[None, :], ly0), 1.0 - wy)
    np.add.at(Wy, (nn, ii[None, :], ly1), wy)
    np.add.at(Wx, (nn, ii[None, :], lx0), 1.0 - wx)
    np.add.at(Wx, (nn, ii[None, :], lx1), wx)

    # per-proposal first arena row and region base
    ab = np.asarray(ARENA_BASE, dtype=np.int64)[lvl]
    row0 = ab + oy.astype(np.int64) * M + ox
    region = np.where(lvl > 0, np.int64(ARENA_BASE[1]),
                      np.minimum(row0 // 16384, 2) * 16384)
    assert (row0 - region >= 0).all()
    assert (row0 - region + (wbuck - 1) * M.astype(np.int64)
            + wbuck - 1 < REGION_W).all()

    wfull = {}
    cls_pos = np.zeros(N, dtype=np.int64)
    for w in BUCKETS:
        ids = np.where(wbuck == w)[0]
        cls_pos[ids] = np.arange(len(ids))
        if len(ids) == 0:
            wfull[w] = np.zeros((0, w * w, S2), dtype=ml_dtypes.bfloat16)
            continue
        wf = np.einsum("niy,njx->nyxij", Wy[ids, :, :w], Wx[ids, :, :w])
        wfull[w] = wf.reshape(len(ids), w * w, S2).astype(ml_dtypes.bfloat16)
    return lvl, wbuck, cls_pos, region, oy, ox, wfull


def _shard(wbuck, region):
    """Round-robin each (region, bucket) class across cores (pad to x8).
    Returns slot_gid [N_CORES, M] and class key tuple."""
    keys = sorted(set(zip(region.tolist(), wbuck.tolist())))
    slot_gid = [[] for _ in range(N_CORES)]
    classes = []
    for r, w in keys:
        ids = np.where((region == r) & (wbuck == w))[0]
        pad = (-len(ids)) % N_CORES
        if pad:
            ids = np.concatenate([ids, np.repeat(ids[-1], pad)])
        per = len(ids) // N_CORES
        for k in range(N_CORES):
            slot_gid[k].extend(ids[k::N_CORES].tolist())
        classes.append((int(r), int(w), per))
    return np.asarray(slot_gid, dtype=np.int64), tuple(classes)


def _plan(classes):
    """Per-core static schedule.
    stripes: (w, region, slot_start, slot_cnt)
    gathers: (region, stripe_start, n_stripes)
    groups:  (stripe_start, n_stripes, slot_start, slot_cnt)"""
    stripes = []
    slot = 0
    for r, w, per in classes:
        left = per
        while left > 0:
            cnt = min(SPB[w], left)
            stripes.append((w, r, slot, cnt))
            slot += cnt
            left -= cnt
    M = slot
    gathers = []
    i = 0
    while i < len(stripes):
        r = stripes[i][1]
        j = i
        while (j < len(stripes) and stripes[j][1] == r
               and j - i < MAX_STR_PER_GATHER):
            j += 1
        gathers.append((r, i, j - i))
        i = j
    groups = []
    for a in range(0, len(stripes), G_ST):
        b = min(a + G_ST, len(stripes))
        s0 = stripes[a][2]
        s1 = stripes[b - 1][2] + stripes[b - 1][3]
        groups.append((a, b - a, s0, s1 - s0))
    return stripes, gathers, groups, M


def _build_graph(classes):
    import concourse.bass as bass
    import concourse.bacc as bacc
    import concourse.mybir as mybir
    import concourse.tile as tile

    stripes, gathers, groups, M = _plan(classes)
    S_tot = len(stripes)
    IDXC = len(gathers) * MAX_STR_PER_GATHER * 8  # 64B-aligned gather blocks

    nc = bacc.Bacc()
    arena = nc.declare_dram_parameter("arena", [ARENA_ROWS, C],
                                      mybir.dt.bfloat16, isOutput=False)
    idxp = nc.declare_dram_parameter("idxp", [128, IDXC],
                                     mybir.dt.int16, isOutput=False)
    wmat = nc.declare_dram_parameter("wmat", [128, S_tot * S2],
                                     mybir.dt.bfloat16, isOutput=False)
    out = nc.declare_dram_parameter("out", [C, M, S2], mybir.dt.bfloat16,
                                    isOutput=True)

    with tile.TileContext(nc) as tc:
        with (
            tc.tile_pool(name="small", bufs=1) as psmall,
            tc.tile_pool(name="slabp", bufs=1) as pslab,
            tc.tile_pool(name="wpool", bufs=3) as pwp,
            tc.tile_pool(name="outp", bufs=3) as po,
            tc.tile_pool(name="ps", bufs=8, space="PSUM") as ppsum,
        ):
            idx_t = psmall.tile([128, IDXC], mybir.dt.int16)
            nc.sync.dma_start(idx_t[:], idxp[:])

            slabs = []        # per gather: (tile, stripe_start)
            for gi, (r, st0, n_str) in enumerate(gathers):
                sl = pslab.tile([128, n_str * C], mybir.dt.bfloat16,
                                tag=f"sl{gi}", name=f"slab_{gi}")
                hi = min(r + REGION_W, ARENA_ROWS)
                c0 = gi * MAX_STR_PER_GATHER * 8
                nc.gpsimd.dma_gather(
                    out_ap=sl[:].rearrange("p (j c) -> p j c", j=n_str),
                    in_ap=arena[r:hi, :],
                    idxs_ap=idx_t[:, c0:c0 + n_str * 8],
                    num_idxs=n_str * 128,
                    num_idxs_reg=n_str * 128,
                    elem_size=C,
                )
                slabs.append((sl, st0))

            def stripe_slab(si):
                for sl, st0 in reversed(slabs):
                    if si >= st0:
                        return sl, si - st0
                raise AssertionError

            def emit_wt(gi):
                a, n_str, s0, n_slots = groups[gi]
                wt = pwp.tile([128, n_str * S2], mybir.dt.bfloat16,
                              tag="wt", name=f"wt_{gi}")
                nc.sync.dma_start(wt[:], wmat[:, a * S2:(a + n_str) * S2])
                return wt

            cast_rr = 0
            wt_next = emit_wt(0)
            for gi, (a, n_str, s0, n_slots) in enumerate(groups):
                wt = wt_next
                if gi + 1 < len(groups):
                    wt_next = emit_wt(gi + 1)
                outAB = po.tile([128, 2 * n_slots * S2], mybir.dt.bfloat16,
                                tag="outAB", name=f"outAB_{gi}")
                # per-slot matmul args within this group
                sargs = []    # (slab, slab_col, wt_col, part_ofs, k)
                for si in range(a, a + n_str):
                    w, r, sst, scnt = stripes[si]
                    sl, j = stripe_slab(si)
                    for q in range(scnt):
                        sargs.append((sl, j * C, (si - a) * S2,
                                      SLOT_OFS[w][q], w * w))
                # one slot per PSUM tile: matmuls with different PE tile
                # positions must not share a PSUM tile (HW wedge)
                for q0 in range(n_slots):
                    psAB = ppsum.tile([128, 512], mybir.dt.float32,
                                      tag="psAB", name=f"ps_{gi}_{q0}")
                    sl, scol, wcol, o, k = sargs[q0]
                    nc.tensor.matmul(psAB[:, 0:S2],
                                     sl[o:o + k, scol:scol + 128],
                                     wt[o:o + k, wcol:wcol + S2],
                                     start=True, stop=True)
                    nc.tensor.matmul(psAB[:, 256:256 + S2],
                                     sl[o:o + k, scol + 128:scol + C],
                                     wt[o:o + k, wcol:wcol + S2],
                                     start=True, stop=True)
                    src = psAB[:].rearrange("p (b x) -> p b x", b=2)[
                        :, :, 0:S2]
                    dst = outAB[:].rearrange("p (b x) -> p b x", b=2)[
                        :, :, q0 * S2:(q0 + 1) * S2]
                    if cast_rr % 2 == 0:
                        nc.vector.tensor_copy(dst, src)
                    else:
                        nc.scalar.copy(dst, src)
                    cast_rr += 1
                nc.sync.dma_start(out[0:128, s0:s0 + n_slots, :],
                                  outAB[:, 0:n_slots * S2])
                nc.scalar.dma_start(out[128:256, s0:s0 + n_slots, :],
                                    outAB[:, n_slots * S2:2 * n_slots * S2])
    nc.finalize()
    return nc


def _prep_core_inputs(k, slot_gid, classes, lvl, cls_pos, region, oy, ox,
                      wfull):
    stripes, gathers, groups, M = _plan(classes)
    S_tot = len(stripes)
    slots = slot_gid[k]
    mw = np.asarray(MAP_HW, dtype=np.int64)
    ab = np.asarray(ARENA_BASE, dtype=np.int64)

    idx = np.zeros((S_tot, 128), dtype=np.int16)
    wblob = np.zeros((128, S_tot * S2), dtype=ml_dtypes.bfloat16)
    for si, (w, r, sst, scnt) in enumerate(stripes):
        K = w * w
        dyx = (np.repeat(np.arange(w), w)[None, :]
               * mw[lvl[slots[sst:sst + scnt]]][:, None]
               + np.tile(np.arange(w), w)[None, :])          # [scnt, K]
        gids = slots[sst:sst + scnt]
        row = (ab[lvl[gids]] + oy[gids].astype(np.int64) * mw[lvl[gids]]
               + ox[gids])[:, None] + dyx - r
        assert row.min() >= 0 and row.max() < REGION_W
        for q in range(scnt):
            o = SLOT_OFS[w][q]
            idx[si, o:o + K] = row[q].astype(np.int16)
            wblob[o:o + K, si * S2:(si + 1) * S2] = wfull[w][cls_pos[gids[q]]]

    # per-gather 64B-aligned int16 blocks: gather gi occupies cols
    # [gi*MAX*8, gi*MAX*8 + n_str*8); within a block index i at
    # [i%16, i//16], replicated 8x down 128 partitions
    IDXC = len(gathers) * MAX_STR_PER_GATHER * 8
    blk = np.zeros((16, IDXC), dtype=np.int16)
    for gi, (r, st0, n_str) in enumerate(gathers):
        c0 = gi * MAX_STR_PER_GATHER * 8
        flat = idx[st0:st0 + n_str].reshape(-1)              # [n_str*128]
        blk[:, c0:c0 + n_str * 8] = flat.reshape(n_str * 8, 16).T
    idxp = np.ascontiguousarray(np.tile(blk, (8, 1)))
    return idxp, wblob


def _install_profile_hook():
    import contextlib
    import ctypes
    import sys
    import types
    if "antenv.axon_hooks" in sys.modules:
        return
    so_path = "/opt/axon/libaxon_pjrt.so"
    try:
        lib = ctypes.CDLL(so_path)
        lib.axon_start_nrt_profile.argtypes = [
            ctypes.POINTER(ctypes.c_int64), ctypes.c_size_t]
        lib.axon_start_nrt_profile.restype = ctypes.c_int64
        lib.axon_stop_nrt_profile.argtypes = [ctypes.c_char_p]
        lib.axon_stop_nrt_profile.restype = ctypes.c_int64
    except (OSError, AttributeError):
        return

    @contextlib.contextmanager
    def _hook(output_dir, device_ids):
        import jax
        jax.devices()
        if device_ids:
            ids = (ctypes.c_int64 * len(device_ids))(*device_ids)
            rc = lib.axon_start_nrt_profile(ids, len(device_ids))
        else:
            rc = lib.axon_start_nrt_profile(None, 0)
        if rc != 0:
            raise RuntimeError(f"axon_start_nrt_profile rc={rc}")
        try:
            yield
        finally:
            n = lib.axon_stop_nrt_profile(str(output_dir).encode())
            if n < 0:
                raise RuntimeError(f"axon_stop_nrt_profile rc={n}")

    mod = types.ModuleType("antenv.axon_hooks")
    mod.get_axon_ntff_profile_hook = lambda: _hook
    mod.set_axon_ntff_profile_hook = lambda h: None
    sys.modules["antenv.axon_hooks"] = mod
    try:
        import antenv
        antenv.axon_hooks = mod
    except ImportError:
        pass


def kernel(f0, f1, f2, f3, proposals):
    global LAST_EXEC_TIME_NS
    try:
        _install_profile_hook()
    except Exception:
        pass
    from concourse.bass_utils import run_bass_kernel_spmd

    feats = (f0, f1, f2, f3)
    N = proposals.shape[0]
    lvl, wbuck, cls_pos, region, oy, ox, wfull = _route_and_weights(
        np.asarray(proposals))
    slot_gid, classes = _shard(wbuck, region)

    if classes not in _GRAPH_CACHE:
        _GRAPH_CACHE[classes] = _build_graph(classes)
    nc = _GRAPH_CACHE[classes]

    arena_np = np.concatenate([
        np.ascontiguousarray(np.asarray(f)[0].transpose(1, 2, 0)).astype(
            ml_dtypes.bfloat16).reshape(-1, C)
        for f in feats
    ], axis=0)
    assert arena_np.shape[0] == ARENA_ROWS

    in_maps = []
    for k in range(N_CORES):
        idxp, wblob = _prep_core_inputs(k, slot_gid, classes, lvl, cls_pos,
                                        region, oy, ox, wfull)
        in_maps.append({"arena": arena_np, "idxp": idxp, "wmat": wblob})

    trace = os.environ.get("KERNEL_TRACE", "0") == "1"
    res = run_bass_kernel_spmd(nc, in_maps, list(range(N_CORES)), trace=trace)
    LAST_EXEC_TIME_NS = res.exec_time_ns

    out_full = np.zeros((N, C, S2), dtype=np.float32)
    for k in range(N_CORES):
        out_full[slot_gid[k]] = res.results[k]["out"].astype(
            np.float32).transpose(1, 0, 2)
    return out_full.reshape(N, C, S, S)
